# revision 32
# baseline (speedup 1.0000x reference)
"""LocationMemoryBank retrieval kernel for 8 Trainium2 NeuronCores.

Strategy (v5): shard the memory table by location id across the 8 cores
(core c owns locs [c*1250, (c+1)*1250)). Queries are routed host-side to the
owning core and deduplicated; each core computes one weighted window-sum per
unique location hit, writing a compact [Urows, 512] result table (bf16). The
final per-query expansion (gather of result rows + fp32 upcast) is the
host-side unshard step.

Device layout: one location per SBUF partition. Unique locs are sorted by
k=min(count,8) descending; tile t gathers only kmax(t) = max k in the tile
window slots per loc, as one contiguous descriptor per partition. All
gathers are issued up front (they fit SBUF simultaneously) so the serial DMA
resource runs gapless. The last tiles' gathers are split into two half-window
segments so their compute starts before the full window lands. The weighted
sum runs as a chain of fused multiply-adds (scalar_tensor_tensor:
acc = g_j * w_j + acc) with per-partition scalar weights on the DVE; the
slot-0 product runs on the Activation engine (per-partition scale) except on
the last active tile. The final chain op writes bf16 directly. Output DMAs
alternate between the SP and Activation sequencers to avoid issue
serialization at the tail.
"""

import os
import sys

import numpy as np

sys.path.insert(0, "/opt/trn_rl_repo")

L, M, D, B = 10000, 20, 512, 16384
K_RECENT = 8
N_CORES = 8
LPC = L // N_CORES          # locations per core

_compiled = {}


def _plan_segments(kmaxs):
    """Per tile: list of (idx_col, slot_lo, slot_hi). The last three active
    tiles are split into smaller gather segments so their weighted-sum chains
    start as soon as each chunk of window data lands, shortening the tail."""
    T_u = len(kmaxs)
    active = [t for t in range(T_u) if kmaxs[t] > 0]
    # Keep total segment count low: each segment costs ~1.07us of Pool
    # descriptor-generation; if the last gens finish after the first
    # output-write's DMA request (~18.6us), outs slip into the gather
    # stream on the serial DMA resource and push the late segments out.
    n_parts = {}
    if len(active) >= 2:
        n_parts[active[-2]] = 2
    if len(active) >= 3:
        n_parts[active[-3]] = 2
    if len(active) >= 4:
        n_parts[active[-4]] = 2
    segments = []
    col = 0
    for t in range(T_u):
        kmax = kmaxs[t]
        if kmax == 0:
            segments.append([])
            continue
        parts = min(n_parts.get(t, 1), kmax)
        # sizes as equal as possible, larger parts first
        base, rem = divmod(kmax, parts)
        sizes = [base + 1] * rem + [base] * (parts - rem)
        segs, lo = [], 0
        for s in sizes:
            segs.append((col, lo, lo + s))
            col += 1
            lo += s
        segments.append(segs)
    return segments, col


def _build_bass(T_u, kmaxs):
    import concourse.bacc as bacc
    import concourse.bass as bass
    import concourse.mybir as mybir
    import concourse.tile as tile

    f32 = mybir.dt.float32
    bf16 = mybir.dt.bfloat16
    i32 = mybir.dt.int32
    mult = mybir.AluOpType.mult
    add = mybir.AluOpType.add

    segments, n_cols = _plan_segments(kmaxs)
    n_segs = sum(len(s) for s in segments)

    # 32KB dynamic-DMA scratch -> 2048-descriptor SWDGE ring, so all gather
    # segments' descriptors fit without await_space stalls on Pool.
    nc = bacc.Bacc(None, dynamic_dma_scratch_size=32768)
    mem = nc.declare_dram_parameter("mem", [LPC * M, D], f32, isOutput=False)
    # idxs[p, c]: flat slot index of segment c's first slot for loc of that row
    idxs = nc.declare_dram_parameter("idxs", [128, n_cols], i32, isOutput=False)
    # wts[p, 8*t+j]: weight of window slot j of loc t*128+p (0 if unused)
    wts = nc.declare_dram_parameter("wts", [128, T_u * 8], f32, isOutput=False)
    out = nc.declare_dram_parameter("out", [T_u * 128, D], bf16, isOutput=True)

    # slot-0 products run on the otherwise-idle Activation engine (per-
    # partition scale); the handoff overlaps the DVE's previous-tile chain.
    # The last active tile stays all-DVE: with earlier tiles' gathers split,
    # the DVE is free before its data lands, so the Act hop would only add
    # latency to the tail. (PE offload was tried and reverted: fp32 diag-
    # matmuls are p-state throttled, fp32r needs producer-side rounding.)
    active = [t for t in range(T_u) if kmaxs[t] > 0]
    act_tiles = set(active[:-1])
    pe_tile = -1

    import contextlib
    from collections import Counter

    with tile.TileContext(nc) as tc:
        with contextlib.ExitStack() as es:
            cpool = es.enter_context(tc.tile_pool(name="const", bufs=1))
            apool = es.enter_context(tc.tile_pool(name="acc", bufs=3))
            opool = es.enter_context(tc.tile_pool(name="out", bufs=max(3, T_u)))
            # one gather pool per segment size, so slots are exact-size
            # (a single shared pool would round every slot up to the largest)
            seg_sizes = Counter(hi - lo for segs in segments for (_, lo, hi) in segs)
            gpools = {
                sz: es.enter_context(tc.tile_pool(name=f"g{sz}", bufs=cnt))
                for sz, cnt in sorted(seg_sizes.items())
            }
            idx_all = cpool.tile([128, n_cols], i32)
            nc.sync.dma_start(out=idx_all[:], in_=idxs[:])
            w_all = cpool.tile([128, T_u * 8], f32)
            nc.scalar.dma_start(out=w_all[:], in_=wts[:])

            # phase A: issue every gather segment up front
            g_tiles = {}   # t -> list of (slot_lo, slot_hi, tile)
            for t in range(T_u):
                g_tiles[t] = []
                for (col, lo, hi) in segments[t]:
                    g_seg = gpools[hi - lo].tile([128, (hi - lo) * D], f32)
                    nc.gpsimd.indirect_dma_start(
                        out=g_seg[:],
                        out_offset=None,
                        in_=mem[:],
                        in_offset=bass.IndirectOffsetOnAxis(
                            ap=idx_all[:, col : col + 1], axis=0
                        ),
                    )
                    g_tiles[t].append((lo, hi, g_seg))

            def g_slot(t, j):
                for lo, hi, g_seg in g_tiles[t]:
                    if lo <= j < hi:
                        return g_seg[:, (j - lo) * D : (j - lo + 1) * D]
                raise AssertionError((t, j))

            # phase B: weighted-sum chains + out writes
            for t in range(T_u):
                kmax = kmaxs[t]
                acc_bf = opool.tile([128, D], bf16)
                if kmax == 0:
                    nc.vector.memset(acc_bf[:], 0.0)
                else:
                    w0 = w_all[:, 8 * t : 8 * t + 1]
                    if kmax == 1:
                        nc.vector.tensor_scalar_mul(acc_bf[:], g_slot(t, 0), w0)
                    else:
                        acc = apool.tile([128, D], f32)
                        if t in act_tiles:
                            nc.scalar.mul(acc[:], g_slot(t, 0), w0)
                        else:
                            nc.vector.tensor_scalar_mul(acc[:], g_slot(t, 0), w0)
                        for j in range(1, kmax):
                            nc.vector.scalar_tensor_tensor(
                                out=acc_bf[:] if j == kmax - 1 else acc[:],
                                in0=g_slot(t, j),
                                scalar=w_all[:, 8 * t + j : 8 * t + j + 1],
                                in1=acc[:],
                                op0=mult,
                                op1=add,
                            )
                # odd tiles (incl. the last, tail-critical one) ride SP:
                # its dge-to-DMA delay is 650ns vs the Act engine's 784ns
                out_eng = nc.sync if t % 2 else nc.scalar
                out_eng.dma_start(out=out[t * 128 : (t + 1) * 128, :], in_=acc_bf[:])

    nc.finalize()
    return nc


def _get_bass(T_u, kmaxs):
    key = ("nc", T_u, tuple(kmaxs))
    if key not in _compiled:
        _compiled[key] = _build_bass(T_u, kmaxs)
    return _compiled[key]


def _host_prep(counts, loc_idx):
    """Route queries to owning shards, dedup + sort by k desc, pack inputs."""
    owner = (loc_idx // LPC).astype(np.int64)              # [B]

    wtab = np.zeros((K_RECENT + 1, K_RECENT), dtype=np.float64)
    for kk in range(1, K_RECENT + 1):
        e = np.exp(np.arange(kk, dtype=np.float64))
        wtab[kk, :kk] = e / e.sum()
    wtab = wtab.astype(np.float32)

    rank_q = np.zeros(B, dtype=np.int64)
    locs_all, n_uniq = [], []
    for c in range(N_CORES):
        sel = np.nonzero(owner == c)[0]
        locs, inv = np.unique(loc_idx[sel], return_inverse=True)
        kl = np.minimum(counts[locs].astype(np.int64), K_RECENT)
        # sort unique locs by k descending (stable: ties keep loc order)
        order = np.argsort(-kl, kind="stable")
        rank_of = np.empty(len(locs), dtype=np.int64)
        rank_of[order] = np.arange(len(locs))
        rank_q[sel] = rank_of[inv]
        locs_all.append(locs[order])
        n_uniq.append(len(locs))
    T_u = max(1, -(-max(n_uniq) // 128))
    urows = T_u * 128

    # kmax per tile must hold across all cores (single SPMD program)
    kmaxs = np.zeros(T_u, dtype=np.int64)
    kls = []
    for c in range(N_CORES):
        locs = locs_all[c]
        kl = np.minimum(counts[locs].astype(np.int64), K_RECENT)
        kls.append(kl)
        km = np.zeros(urows, dtype=np.int64)
        km[: len(locs)] = kl
        kmaxs = np.maximum(kmaxs, km.reshape(T_u, 128).max(axis=1))
    kmaxs_l = [int(k) for k in kmaxs]
    segments, n_cols = _plan_segments(kmaxs_l)

    idxs_all, wts_all = [], []
    for c in range(N_CORES):
        locs = locs_all[c]
        cl = counts[locs].astype(np.int64)
        kl = kls[c]
        st = cl - kl
        ssl = np.zeros(urows, dtype=np.int64)
        ssl[: len(locs)] = (locs.astype(np.int64) - c * LPC) * M + st
        wl = np.zeros((urows, K_RECENT), dtype=np.float32)
        wl[: len(locs)] = wtab[kl]

        # row r = t*128 + p -> idx[p, t], wts[p, 8t+j]
        ss = ssl.reshape(T_u, 128).T                         # [128, T_u]
        ww = wl.reshape(T_u, 128, K_RECENT).transpose(1, 0, 2)  # [128, T_u, 8]

        idx_cols = np.zeros((128, n_cols), dtype=np.int32)
        for t in range(T_u):
            for (col, lo, hi) in segments[t]:
                idx_cols[:, col] = ss[:, t] + lo
        idxs_all.append(np.ascontiguousarray(idx_cols))
        wts_all.append(np.ascontiguousarray(ww.reshape(128, T_u * 8)))

    return idxs_all, wts_all, kmaxs_l, T_u, owner, rank_q


def kernel(memory_feats, counts, loc_idx):
    from concourse.bass_utils import run_bass_kernel_spmd

    memory_feats = np.ascontiguousarray(memory_feats, dtype=np.float32)
    counts = np.asarray(counts, dtype=np.int32)
    loc_idx = np.asarray(loc_idx, dtype=np.int32)

    idxs_all, wts_all, kmaxs, T_u, owner, rank_q = _host_prep(counts, loc_idx)
    nc = _get_bass(T_u, kmaxs)

    in_maps = [
        {
            "mem": memory_feats[c * LPC : (c + 1) * LPC].reshape(LPC * M, D),
            "idxs": idxs_all[c],
            "wts": wts_all[c],
        }
        for c in range(N_CORES)
    ]
    trace = bool(int(os.environ.get("KERNEL_TRACE", "0")))
    res = run_bass_kernel_spmd(nc, in_maps, list(range(N_CORES)), trace=trace)
    _compiled["last_results"] = res
    res_stack = np.stack([res.results[c]["out"] for c in range(N_CORES)])
    return np.ascontiguousarray(res_stack[owner, rank_q].astype(np.float32))
